# revision 5
# baseline (speedup 1.0000x reference)
"""KMaxPool1d (top-k=8 along last dim, positional order) on 8 trn2 NeuronCores.

Contract: kernel(**inputs) takes the FULL inputs
    inputs: [32, 512, 4096] float32
    top_k:  scalar (== 8)
and returns the FULL output [32, 512, 8] float32, equal to
    jnp.take_along_axis(inputs, jnp.sort(jax.lax.top_k(inputs, 8)[1], -1), -1)

Strategy: pure data parallel over rows. The (32, 512) leading dims flatten to
16384 independent rows of 4096; each of the 8 cores gets a contiguous slab of
2048 rows = 16 tiles of [128 partitions x 4096].

Per tile, on the DVE:
  max        -> top-8 values, descending                    (full scan)
  max_index  -> their positions; duplicate values match
                successive occurrences, which reproduces
                jax.lax.top_k's lowest-index-first tie-break (full scan)
  -idx, max  -> positions sorted ascending (8-wide sort via max8 of negations)
  eq-match   -> out[:, j] = sum_r (idx_sorted[j] == idx[r]) * vals[r]
                (indices are distinct, so exactly one term fires)
"""

import sys

if "/opt/trn_rl_repo" not in sys.path:
    sys.path.insert(0, "/opt/trn_rl_repo")

import numpy as np

B, C, L, K = 32, 512, 4096, 8
N_CORES = 8
ROWS = B * C
ROWS_PER_CORE = ROWS // N_CORES  # 2048

_NC_CACHE = {}


def _build_nc(rows_per_core=ROWS_PER_CORE):
    import concourse.bass as bass
    import concourse.bacc as bacc
    import concourse.mybir as mybir
    from concourse.tile import TileContext

    F32 = mybir.dt.float32
    U32 = mybir.dt.uint32

    # Bacc (not plain Bass): its compile() pass splits multi-sem waits into
    # event-semaphore nops — walrus rejects >1 sync wait per instruction.
    nc = bacc.Bacc(None)
    x = nc.dram_tensor("x", [rows_per_core, L], F32, kind="ExternalInput")
    y = nc.dram_tensor("y", [rows_per_core, K], F32, kind="ExternalOutput")
    ntiles = rows_per_core // 128

    with TileContext(nc) as tc:
        with (
            # bufs=8 with exactly one DMA per tile keeps slot reuse on the
            # same SWDGE queue (Tile round-robins 8 queues), so each load
            # needs at most one semaphore wait — the DIRECT2D DMA struct
            # can't encode more.
            tc.tile_pool(name="xp", bufs=8) as xp,
            tc.tile_pool(name="sp", bufs=4) as sp,
            tc.tile_pool(name="op", bufs=1) as op,
        ):
            out_all = op.tile([128, ntiles, K], F32)
            for t in range(ntiles):
                xt = xp.tile([128, L], F32, tag="xt")
                nc.gpsimd.dma_start(xt[:], x[bass.ts(t, 128), :])

                vals = sp.tile([128, K], F32, tag="vals")
                nc.vector.max(vals[:], xt[:])

                idx = sp.tile([128, K], U32, tag="idx")
                nc.vector.max_index(idx[:], vals[:], xt[:])

                nidx = sp.tile([128, K], F32, tag="nidx")
                nc.vector.tensor_scalar_mul(nidx[:], idx[:], -1.0)

                srt = sp.tile([128, K], F32, tag="srt")
                nc.vector.max(srt[:], nidx[:])

                eq = sp.tile([128, K, K], F32, tag="eq")
                a = srt[:].rearrange("p (j o) -> p j o", o=1).to_broadcast([128, K, K])
                b = nidx[:].rearrange("p (o r) -> p o r", o=1).to_broadcast([128, K, K])
                v = vals[:].rearrange("p (o r) -> p o r", o=1).to_broadcast([128, K, K])
                nc.vector.tensor_tensor(eq[:], a, b, op=mybir.AluOpType.is_equal)
                nc.vector.tensor_tensor(eq[:], eq[:], v, op=mybir.AluOpType.mult)

                nc.vector.tensor_reduce(
                    out_all[:, t, :],
                    eq[:],
                    axis=mybir.AxisListType.X,
                    op=mybir.AluOpType.add,
                )
            # one store for all tiles: y[(t p) k] <- out_all[p, t, k]
            nc.gpsimd.dma_start(
                y.rearrange("(t p) k -> p t k", p=128), out_all[:]
            )
    nc.finalize()  # runs Bacc.compile(): reg alloc + sync-wait splitting
    return nc


def _get_nc():
    if "nc" not in _NC_CACHE:
        _NC_CACHE["nc"] = _build_nc()
    return _NC_CACHE["nc"]


def run_spmd(flat_x, trace=False):
    """flat_x: [16384, 4096] f32. Returns ([16384, 8] f32, exec_time_ns|None)."""
    from concourse.bass_utils import run_bass_kernel_spmd

    nc = _get_nc()
    shards = np.split(np.ascontiguousarray(flat_x), N_CORES, axis=0)
    res = run_bass_kernel_spmd(
        nc,
        [{"x": s} for s in shards],
        list(range(N_CORES)),
        trace=trace,
    )
    out = np.concatenate([res.results[c]["y"] for c in range(N_CORES)], axis=0)
    return out, res.exec_time_ns


def kernel(inputs, top_k):
    assert int(top_k) == K, f"kernel hardcodes top_k={K}, got {top_k}"
    x = np.asarray(inputs, dtype=np.float32).reshape(ROWS, L)
    out, _ = run_spmd(x)
    return out.reshape(B, C, K)
